# revision 27
# baseline (speedup 1.0000x reference)
"""v4: bf16 moving operands + consolidated DMA + schedule rebalance.
Core algorithm as v3 (head-sharded across 8 cores: DPC=128 dims = 2 heads
per core; per-core QKV projection -> per-head attention with ones-row
denominator -> out projection partial, host-side all-reduce).

275.1us -> 225.9us (TimelineSim). Changes vs v3:
 - moving matmul operands bf16 (qt, QT, exp tiles, wo) + bf16 out_part:
   halves DMA wire + SBUF for those; stationary operands stay f32r
   (self-loading); PSUM fp32; host accumulates partials in fp32
 - qt input pre-transposed AND converted to bf16 on host (free host prep)
 - DMA count cut ~40%: [128,1024] qt tiles, one strided DMA per weight
   half; HWDGE + the DMA wire are single shared serial devices in the
   cost model (~630+728ns per transfer), so fewer/bigger transfers win
 - prologue: by-k-tile projection (K/Q/V accumulate as each qt tile
   lands), DMA issue order matched to PE consumption order
 - biases pre-scaled on host (b' = b*s), packed [DPC, 3], evacuations
   fold (ps*s + b') via DVE tensor_scalar or ACT Identity-activation
 - outproj of batch0 interleaved with batch1's projections (program-order
   interleave keeps the in-order PE queue busy through evac/store stalls);
   evacs alternate ACT/DVE; stores rotate 3 DMA queues
 - attention end-copies + normalize split into 512-col halves so the
   final outproj starts as soon as its half is normalized; denominator
   broadcast via DRAM roundtrip (DMA row-broadcast)
 - remaining structure (TimelineSim): ACT is saturated ~66.5us per batch
   during attention (128 exps/batch are the hard ACT floor); PE busy
   ~168us is the global floor. fp8 DoubleRow would halve ctx/proj PE time
   but fails the 2e-2 gate (e4m3 quantization ~3.6% rms -> ~2.5e-2).
"""

import functools
from collections import deque
from contextlib import ExitStack

import numpy as np
import ml_dtypes

import concourse.bass as bass
import concourse.tile as tile
from concourse import mybir
from concourse.bass_utils import run_bass_kernel_spmd

B, S, D, H, DH = 2, 2048, 1024, 16, 64
N_CORES = 8
DPC = D // N_CORES
BS = B * S
NQC = S // 1024           # 2
NST = S // 128            # 16
NKT = D // 128            # 8

F32 = mybir.dt.float32
F32R = mybir.dt.float32r
BF16 = mybir.dt.bfloat16
Act = mybir.ActivationFunctionType
Alu = mybir.AluOpType
BF16NP = ml_dtypes.bfloat16


def _split_sync_commands(nc, max_waits=1, max_updates=8):
    for fn in nc.m.functions:
        for bb in fn.blocks:
            new_insts = []
            changed = False
            for inst in bb.instructions:
                si = getattr(inst, "sync_info", None)
                if si is not None:
                    waits = list(si.on_wait or [])
                    if len(waits) > max_waits:
                        for w in waits[:-max_waits]:
                            new_insts.append(mybir.InstNoOp(
                                name=nc.get_next_instruction_name(),
                                ins=[], outs=[], engine=inst.engine,
                                sync_info=mybir.SyncInfo(on_wait=[w], on_update=[]),
                            ))
                        si.on_wait = waits[-max_waits:]
                        changed = True
                    updates = list(si.on_update or [])
                    if len(updates) > max_updates:
                        si.on_update = updates[:max_updates]
                        new_insts.append(inst)
                        new_insts.append(mybir.InstNoOp(
                            name=nc.get_next_instruction_name(),
                            ins=[], outs=[], engine=inst.engine,
                            sync_info=mybir.SyncInfo(
                                on_wait=[], on_update=updates[max_updates:]),
                        ))
                        changed = True
                        continue
                new_insts.append(inst)
            if changed:
                bb.instructions = new_insts


def _bcast_rows(ap, nrows):
    return bass.AP(tensor=ap.tensor, offset=ap.offset,
                   ap=[[0, nrows]] + [list(p) for p in ap.ap[1:]])


def _strided(ap, pattern, offset=0):
    return bass.AP(tensor=ap.tensor, offset=ap.offset + offset,
                   ap=[list(p) for p in pattern])


@functools.lru_cache(maxsize=1)
def _build():
    nc = bass.Bass()
    qt_d = nc.dram_tensor("qt", [D, BS], BF16, kind="ExternalInput")
    wq_d = nc.dram_tensor("wq", [D, DPC], BF16, kind="ExternalInput")
    wk_d = nc.dram_tensor("wk", [D, DPC], BF16, kind="ExternalInput")
    wv_d = nc.dram_tensor("wv", [D, DPC], BF16, kind="ExternalInput")
    bias_d = nc.dram_tensor("bias", [DPC, 3], F32, kind="ExternalInput")
    wo_d = nc.dram_tensor("wo", [DPC, D], BF16, kind="ExternalInput")
    out_d = nc.dram_tensor("out_part", [BS, D], BF16, kind="ExternalOutput")
    dn_d = nc.dram_tensor("dn_scratch", [2, S], F32)
    ident_d = nc.inline_tensor(np.eye(128, dtype=np.float32), "ident")
    ones_d = nc.inline_tensor(np.ones((1, 1), dtype=BF16NP), "ones_const")

    with tile.TileContext(nc) as tc, ExitStack() as ctx:
        consts = ctx.enter_context(tc.tile_pool(name="consts", bufs=1))
        qt_pool = ctx.enter_context(tc.tile_pool(name="qt", bufs=2))
        proj = ctx.enter_context(tc.tile_pool(name="proj", bufs=2))
        vpool = ctx.enter_context(tc.tile_pool(name="vpool", bufs=2))
        vtp = ctx.enter_context(tc.tile_pool(name="vtp", bufs=1))
        ctxp = ctx.enter_context(tc.tile_pool(name="ctxp", bufs=2))
        expp = ctx.enter_context(tc.tile_pool(name="expp", bufs=12))
        dnp = ctx.enter_context(tc.tile_pool(name="dnp", bufs=1))
        outp = ctx.enter_context(tc.tile_pool(name="outp", bufs=6))
        psp = ctx.enter_context(tc.tile_pool(name="psp", bufs=1, space="PSUM"))

        def ps_tile(shape, tag, dtype=F32):
            return psp.tile(shape, dtype, tag=tag, name="ps_" + tag)

        # ---- constants: one strided DMA per tensor on the Pool queue ------
        wk_sb = consts.tile([128, NKT, DPC], BF16, tag="wk")
        wq_sb = consts.tile([128, NKT, DPC], BF16, tag="wq")
        wv_sb = consts.tile([128, NKT, DPC], BF16, tag="wv")
        wpat = [[DPC, 128], [128 * DPC, NKT], [1, DPC]]
        wpat_h = [[DPC, 128], [128 * DPC, NKT // 2], [1, DPC]]
        nc.sync.dma_start(out=wk_sb[:, 0:NKT // 2, :],
                          in_=_strided(wk_d[:, :], wpat_h))
        bias_sb = consts.tile([128, 3], F32, tag="bias")
        nc.gpsimd.dma_start(out=bias_sb, in_=bias_d[:, :])
        ident_sb = consts.tile([128, 128], F32, tag="ident")
        nc.gpsimd.dma_start(out=ident_sb, in_=ident_d[:, :])
        wo_sb = consts.tile([128, D], BF16, tag="wo")
        nc.gpsimd.dma_start(out=wo_sb, in_=wo_d[:, :])
        eighth_sb = consts.tile([128, 1], F32, tag="eighth")
        nc.vector.memset(eighth_sb, 0.125)
        one_sb = consts.tile([128, 1], F32, tag="one")
        nc.vector.memset(one_sb, 1.0)
        zero_sb = consts.tile([128, 1], F32, tag="zero")
        nc.vector.memset(zero_sb, 0.0)

        state = {}

        def load(b, engines):
            """qt load: 16 DMAs of [128, 1024] bf16, pc-major so chunk-0
            projections unblock first. Never put scalar-queue (ACT-issued)
            DMAs where the ACT engine is busy."""
            qt_sb = qt_pool.tile([128, NKT, S], BF16, tag="qt", name="qt")
            i = 0
            for pc in range(2):
                for k in range(NKT):
                    engines[i % len(engines)].dma_start(
                        out=qt_sb[:, k, pc * 1024:(pc + 1) * 1024],
                        in_=qt_d[k * 128:(k + 1) * 128,
                                 b * S + pc * 1024: b * S + (pc + 1) * 1024])
                    i += 1
            state[b, "qt"] = qt_sb

        def proj_all_by_k(b, pc, tags, evacs):
            """K/Q/V projections for one pc chunk, interleaved by k-tile so
            the PE keeps pace with qt DMA arrivals (shared-wire order)."""
            qt_sb = state[b, "qt"]
            pss = {w: ps_tile([128, 1024], t) for w, t in zip("kqv", tags)}
            for k in range(NKT):
                for which in "kqv":
                    w_sl = {"q": wq_sb, "k": wk_sb, "v": wv_sb}[which][:, k, :]
                    for hh in range(2):
                        nc.tensor.matmul(
                            pss[which][:, hh * 512:(hh + 1) * 512], w_sl,
                            qt_sb[:, k, pc * 1024 + hh * 512:
                                  pc * 1024 + (hh + 1) * 512],
                            start=(k == 0), stop=(k == NKT - 1))
            for which, evac_act in zip("kqv", evacs):
                col = {"q": 0, "k": 1, "v": 2}[which]
                sc = 0.125 if which == "q" else 1.0
                b_ap = bias_sb[:, col:col + 1]
                dst = state[b, {"q": "QT", "k": "KT", "v": "VT"}[which]]
                out_sl = dst[:, pc * 1024:(pc + 1) * 1024]
                if evac_act:
                    nc.scalar.activation(out_sl, pss[which], Act.Identity,
                                         bias=b_ap, scale=sc)
                else:
                    nc.vector.tensor_scalar(
                        out=out_sl, in0=pss[which],
                        scalar1=sc, scalar2=b_ap, op0=Alu.mult, op1=Alu.add)

        def proj_chunk(b, which, pc, tag, evac_act=False):
            qt_sb = state[b, "qt"]
            col = {"q": 0, "k": 1, "v": 2}[which]
            sc = 0.125 if which == "q" else 1.0
            b_ap = bias_sb[:, col:col + 1]
            dst = state[b, {"q": "QT", "k": "KT", "v": "VT"}[which]]
            ps = ps_tile([128, 1024], tag)
            for k in range(NKT):
                w_sl = {"q": wq_sb, "k": wk_sb, "v": wv_sb}[which][:, k, :]
                for hh in range(2):
                    nc.tensor.matmul(
                        ps[:, hh * 512:(hh + 1) * 512], w_sl,
                        qt_sb[:, k, pc * 1024 + hh * 512: pc * 1024 + (hh + 1) * 512],
                        start=(k == 0), stop=(k == NKT - 1))
            out_sl = dst[:, pc * 1024:(pc + 1) * 1024]
            if evac_act:
                nc.scalar.activation(out_sl, ps, Act.Identity, bias=b_ap, scale=sc)
            else:
                nc.vector.tensor_scalar(
                    out=out_sl, in0=ps,
                    scalar1=sc, scalar2=b_ap, op0=Alu.mult, op1=Alu.add)

        def alloc_proj(b):
            state[b, "QT"] = proj.tile([128, S], BF16, tag="QT", name="QT")
            state[b, "KT"] = proj.tile([128, S], BF16, tag="KT", name="KT")
            state[b, "VT"] = vtp.tile([128, S], F32, tag="VT", name="VT")

        def alloc_v(b):
            V = vpool.tile([128, NST, 2, DH + 1], BF16, tag="V", name="V")
            ones_ap = ones_d[:, :]
            nc.sync.dma_start(
                out=V[:, :, :, DH:DH + 1],
                in_=bass.AP(tensor=ones_ap.tensor, offset=ones_ap.offset,
                            ap=[[0, 128], [0, NST * 2], [1, 1]]))
            state[b, "V"] = V

        def tr_one(b, st, tag):
            VT, V = state[b, "VT"], state[b, "V"]
            ps_t = ps_tile([128, 128], tag)
            nc.tensor.transpose(ps_t, VT[:, st * 128:(st + 1) * 128], ident_sb)
            nc.vector.tensor_copy(V[:, st, :, 0:DH], ps_t[:, :])

        def outproj_st(b, st, tag, evac_act=False, store_eng=None):
            ctxT = state[b, "ctxT"]
            o_sb = outp.tile([128, D], BF16, tag="o", name="o_sb")
            ps = ps_tile([128, 1024], tag)
            for oc in range(2):
                nc.tensor.matmul(ps[:, oc * 512:(oc + 1) * 512],
                                 ctxT[:, st * 128:(st + 1) * 128],
                                 wo_sb[:, oc * 512:(oc + 1) * 512],
                                 start=True, stop=True)
            if evac_act:
                nc.scalar.activation(o_sb, ps, Act.Copy, bias=0.0, scale=1.0)
            else:
                nc.vector.tensor_copy(o_sb, ps)
            eng = store_eng or (nc.sync, nc.gpsimd, nc.scalar)[st % 3]
            eng.dma_start(
                out=out_d[b * S + st * 128: b * S + (st + 1) * 128, :], in_=o_sb)

        def alloc_attn(b):
            state[b, "ctxT"] = ctxp.tile([128, S], BF16, tag="ctxT", name="ctxT")
            state[b, "denom"] = dnp.tile([1, 2, S], F32, tag="denom", name="denom")

        def attention_qc(b, qc, inserts=()):
            QT, KT, V = state[b, "QT"], state[b, "KT"], state[b, "V"]
            ctxT, denom = state[b, "ctxT"], state[b, "denom"]
            sl = slice(qc * 1024, (qc + 1) * 1024)
            inserts = deque(inserts)
            pcs = [None, None]
            pss = [None, None]
            pending = deque()

            def scores(u, sk):
                pss[u] = ps_tile([128, 1024], "sA" if u == 0 else "sB")
                for hh in range(2):
                    nc.tensor.matmul(
                        pss[u][:, hh * 512:(hh + 1) * 512],
                        KT[u * DH:(u + 1) * DH, sk * 128:(sk + 1) * 128],
                        QT[u * DH:(u + 1) * DH,
                           qc * 1024 + hh * 512:qc * 1024 + (hh + 1) * 512],
                        start=True, stop=True)

            def expop(u, sk):
                e = expp.tile([128, 1024], BF16, tag="exp", name="exp_t")
                nc.scalar.activation(e, pss[u], Act.Exp, bias=zero_sb, scale=1.0)
                pending.append((u, sk, e))

            def ctx_drain(target_len):
                while len(pending) > target_len:
                    u, sk, e = pending.popleft()
                    if pcs[u] is None:
                        pcs[u] = ps_tile([DH + 1, 1024], "cA" if u == 0 else "cB")
                    for hh in range(2):
                        nc.tensor.matmul(
                            pcs[u][:, hh * 512:(hh + 1) * 512], V[:, sk, u, :],
                            e[:, hh * 512:(hh + 1) * 512],
                            start=(sk == 0), stop=(sk == NST - 1))

            scores(0, 0)
            scores(1, 0)
            for sk in range(NST):
                expop(0, sk)
                expop(1, sk)
                if sk + 1 < NST:
                    scores(0, sk + 1)
                if inserts:
                    inserts.popleft()()
                if sk + 1 < NST:
                    scores(1, sk + 1)
                if inserts:
                    ctx_drain(12)
                else:
                    ctx_drain(6)
            while inserts:
                inserts.popleft()()
            ctx_drain(0)

            for h in range(2):
                hsl = slice(qc * 1024 + h * 512, qc * 1024 + (h + 1) * 512)
                psl = slice(h * 512, (h + 1) * 512)
                for u in range(2):
                    nc.vector.tensor_copy(ctxT[u * DH:(u + 1) * DH, hsl],
                                          pcs[u][0:DH, psl])
                    nc.vector.tensor_copy(denom[0:1, u, hsl],
                                          pcs[u][DH:DH + 1, psl])
                nc.sync.dma_start(out=dn_d[:, hsl], in_=denom[0:1, :, hsl])

        def normalize(b, qc=None, half=None):
            ctxT, denom = state[b, "ctxT"], state[b, "denom"]
            if half is None:
                sl = slice(0, S) if qc is None else slice(qc * 1024, (qc + 1) * 1024)
            else:
                sl = slice(qc * 1024 + half * 512, qc * 1024 + (half + 1) * 512)
            key = (b, "rep")
            if key not in state:
                state[key] = dnp.tile([128, S], F32, tag="rep", name="rep")
            rep = state[key]
            for u in range(2):
                nc.sync.dma_start(out=rep[u * DH:(u + 1) * DH, sl],
                                  in_=_bcast_rows(dn_d[u:u + 1, sl], DH))
            kb = (b, "repb")
            if kb not in state:
                state[kb] = dnp.tile([128, S], BF16, tag="repb", name="repb")
            repb = state[kb]
            nc.vector.reciprocal(rep[:, sl], rep[:, sl])
            nc.vector.tensor_copy(repb[:, sl], rep[:, sl])
            nc.vector.tensor_mul(ctxT[:, sl], ctxT[:, sl], repb[:, sl])

        def thunk(f, *a):
            def g():
                f(*a)
            return g

        # =========================== schedule ===========================
        # prologue DMA order on the shared wire: wk halves, then qt-pc0
        # tiles interleaved with wq/wv halves; everything else behind.
        qt_sb0 = qt_pool.tile([128, NKT, S], BF16, tag="qt", name="qt")
        state[0, "qt"] = qt_sb0

        def qt_dma(eng, k, pc):
            eng.dma_start(
                out=qt_sb0[:, k, pc * 1024:(pc + 1) * 1024],
                in_=qt_d[k * 128:(k + 1) * 128, pc * 1024:(pc + 1) * 1024])

        def w_half(eng, w_sb, w_d, h):
            eng.dma_start(out=w_sb[:, h * (NKT // 2):(h + 1) * (NKT // 2), :],
                          in_=_strided(w_d[:, :], wpat_h,
                                       offset=h * (NKT // 2) * 128 * DPC))

        # issue plan (wire alternates queues): wk0,qt0,wq0,wv0,qt1..qt4,
        # wk1,wv1,wq1,qt5..qt7 — h1 weight halves land just before PE's k=4
        qt_dma(nc.scalar, 0, 0)
        w_half(nc.sync, wq_sb, wq_d, 0)
        w_half(nc.scalar, wv_sb, wv_d, 0)
        qt_dma(nc.sync, 1, 0)
        qt_dma(nc.scalar, 2, 0)
        qt_dma(nc.sync, 3, 0)
        qt_dma(nc.scalar, 4, 0)
        w_half(nc.sync, wk_sb, wk_d, 1)
        w_half(nc.scalar, wv_sb, wv_d, 1)
        w_half(nc.sync, wq_sb, wq_d, 1)
        qt_dma(nc.scalar, 5, 0)
        qt_dma(nc.sync, 6, 0)
        qt_dma(nc.scalar, 7, 0)
        for k in range(NKT):
            qt_dma((nc.sync, nc.scalar)[k % 2], k, 1)

        alloc_proj(0)
        alloc_v(0)
        proj_all_by_k(0, 0, ("sA", "sB", "cA"), (False, True, False))
        proj_all_by_k(0, 1, ("sA", "sB", "cA"), (False, True, False))
        for st in range(NST):
            tr_one(0, st, ("sA", "sB", "cA", "cB")[st % 4])
        load(1, (nc.sync, nc.gpsimd))  # overlaps attn0; ACT queue untouched

        alloc_attn(0)
        attention_qc(0, 0)
        normalize(0, 0)       # overlaps attn0-qc1
        attention_qc(0, 1)
        normalize(0, 1)

        alloc_proj(1)
        alloc_v(1)
        proj_chunk(1, "k", 0, "sA")
        proj_chunk(1, "q", 0, "sB")
        outproj_st(0, 0, "cA", evac_act=True)
        outproj_st(0, 1, "cB")
        proj_chunk(1, "v", 0, "sA")
        outproj_st(0, 2, "cA", evac_act=True)
        outproj_st(0, 3, "cB")
        for st in range(8):
            tr_one(1, st, "cA" if st % 2 == 0 else "cB")
        outproj_st(0, 4, "cA", evac_act=True)
        outproj_st(0, 5, "cB")
        proj_chunk(1, "k", 1, "sB")
        outproj_st(0, 6, "cA", evac_act=True)
        outproj_st(0, 7, "cB")
        proj_chunk(1, "q", 1, "sA")
        outproj_st(0, 8, "cA", evac_act=True)
        outproj_st(0, 9, "cB")
        proj_chunk(1, "v", 1, "sB")
        outproj_st(0, 10, "cA", evac_act=True)
        outproj_st(0, 11, "cB")
        for st in range(8, NST):
            tr_one(1, st, "cA" if st % 2 == 0 else "cB")
        for st in range(12, NST):
            outproj_st(0, st, ("cA", "cB")[st % 2], evac_act=(st % 2 == 0))

        alloc_attn(1)
        attention_qc(1, 0)
        normalize(1, 0)       # overlaps attn1-qc1 (DVE/DMA only, no PE)
        attention_qc(1, 1)
        normalize(1, 1, half=0)
        for st in range(4):
            outproj_st(1, st, ("cA", "cB")[st % 2], evac_act=(st % 2 == 0))
        normalize(1, 1, half=1)
        for st in range(4, 8):
            outproj_st(1, st, ("cA", "cB")[st % 2], evac_act=(st % 2 == 0))
        for st in range(8, 12):
            outproj_st(1, st, ("cA", "cB", "sA", "sB")[st % 4],
                       evac_act=(st % 2 == 1))
        for st in range(12, NST):
            outproj_st(1, st, ("cA", "cB", "sA", "sB")[st % 4],
                       evac_act=(st % 2 == 1),
                       store_eng=(nc.sync, nc.gpsimd, nc.scalar, nc.gpsimd)[st % 4])

    _split_sync_commands(nc)
    return nc


def _prepare(query, q_w, q_b, k_w, k_b, v_w, v_b, out_w):
    qt = np.ascontiguousarray(query.reshape(BS, D).T).astype(BF16NP)  # [D, BS]
    in_maps = []
    for c in range(N_CORES):
        sl = slice(c * DPC, (c + 1) * DPC)
        bias = np.stack([q_b[sl] * 0.125, k_b[sl], v_b[sl]], axis=1)
        in_maps.append({
            "qt": qt,
            "wq": np.ascontiguousarray(q_w[sl, :].T).astype(BF16NP),
            "wk": np.ascontiguousarray(k_w[sl, :].T).astype(BF16NP),
            "wv": np.ascontiguousarray(v_w[sl, :].T).astype(BF16NP),
            "bias": np.ascontiguousarray(bias.astype(np.float32)),
            "wo": np.ascontiguousarray(out_w[:, sl].T).astype(BF16NP),
        })
    return in_maps


def kernel(query, mask, q_w, q_b, k_w, k_b, v_w, v_b, out_w, out_b):
    query = np.asarray(query, dtype=np.float32)
    q_w = np.asarray(q_w, dtype=np.float32); q_b = np.asarray(q_b, dtype=np.float32)
    k_w = np.asarray(k_w, dtype=np.float32); k_b = np.asarray(k_b, dtype=np.float32)
    v_w = np.asarray(v_w, dtype=np.float32); v_b = np.asarray(v_b, dtype=np.float32)
    out_w = np.asarray(out_w, dtype=np.float32); out_b = np.asarray(out_b, dtype=np.float32)

    in_maps = _prepare(query, q_w, q_b, k_w, k_b, v_w, v_b, out_w)
    nc = _build()
    res = run_bass_kernel_spmd(nc, in_maps, core_ids=list(range(N_CORES)))
    out = np.zeros((BS, D), dtype=np.float32)
    for c in range(N_CORES):
        out += res.results[c]["out_part"].astype(np.float32)
    out += out_b[None, :]
    return out.reshape(B, S, D)
